# revision 28
# baseline (speedup 1.0000x reference)
"""AdaptiveConv2d Trainium2 kernel (8 NeuronCores, data-parallel over batch).

Per core: 2 samples. On-device per sample:
  MLP z->20->30->36928 generates per-sample 3x3x64x64 conv kernels + biases,
  then 3x3 SAME conv (64->64ch, 128x128) + bias + relu.

Conv scheme ("M=128 row-pair packing", 75% PE util):
  x is staged host-side into a zero-padded (130-wide rows) stacked bf16
  layout X2 = [xpad ; xpad shifted +130] on 128 partitions (one contiguous
  full-width DMA per sample).  For each 512-pixel output tile g and each
  dj in {0,1,2} one bf16 matmul with lhsT L_dj [128,128] covers taps
  (di=-1,dj),(di=0,dj) for tile g (PSUM rows 0:64) and tap (di=+1,dj) for
  tile positions g*512-130 (PSUM rows 64:128).
  Merge pipeline (one 512-wide pass per engine per group):
    ACT:    fold-copy PSUM[64:128] -> SBUF tb (cross-partition copy)
    DVE:    out_pre = PSUM[0:64] + tb window(s)
    GPSIMD: out_sb = max(out_pre + bias, 0) (tensor_scalar, bf16 out)

MLP big layer runs as one bf16 GEMM with host-prepermuted W2 so that the
output lands in SBUF in a layout from which the six lhsT tiles (2 samples x
3 dj) are built with 24 contiguous SBUF->SBUF DMAs.

NOTE: this container's walrus accepts at most ONE sync wait per
instruction; _split_multiwaits() legalizes the Tile-scheduled module.
"""

import os
import numpy as np
import ml_dtypes

B, C_IN, C_OUT, KS, H, W_DIM, F_IN = 16, 64, 64, 3, 128, 128, 16
NKW = C_OUT * C_IN * KS * KS  # 36864
N_CORES = 8
SPC = B // N_CORES  # samples per core = 2

RW = W_DIM + 2          # padded row width = 130
NPAD = RW * (H + 2)     # 16900 padded positions (image base IB=1)
IB = 1
OB = IB + RW            # first output position = 131
NOUT = 16640            # out_sb width (indices 0..16638 used)
NTILE = 512
NG = 33
SPLIT = NTILE - RW      # 382
NW2 = 12352             # GEMM width: 3*32*128 kernels + 64 biases
KDIM = 124              # 4 chunks x 30 + 4 bias rows
Y_CHUNKS = [(0, 31), (31, 63), (63, 94), (94, 128)]  # rows per output DMA

LAST_EXEC_NS = None
TRACE = os.environ.get("BASS_KERNEL_TRACE", "0") == "1"

_NC_CACHE = None


def _build_w2p(W2, b2):
    """[124, 12352] f32: pre-permuted W2.T so GEMM output = L-tile layout."""
    W2P = np.zeros((KDIM, NW2), np.float32)
    dj, pi, mb, co = np.meshgrid(
        np.arange(3), np.arange(32), np.arange(2), np.arange(64), indexing="ij"
    )
    n = (dj * 32 + pi) * 128 + mb * 64 + co
    for j in range(4):
        p = 32 * j + pi
        kb = p // 64
        ci = p % 64
        di = np.where(mb == 0, kb, 2)
        valid = ~((mb == 1) & (kb == 0))
        m = co * 576 + ci * 9 + di * 3 + dj
        nv = n[valid]
        mv = m[valid]
        W2P[30 * j : 30 * j + 30, nv] = W2[mv, :].T
        W2P[120 + j, nv] = b2[mv]
    cot = np.arange(64)
    W2P[0:30, 12288 + cot] = W2[36864 + cot, :].T
    W2P[120, 12288 + cot] = b2[36864 + cot]
    return W2P


def _build_x2_host(x_pair):
    """[SPC, 128, NPAD] bf16: padded image + its (+130)-shifted copy stacked
    on the partition axis, zeros baked in."""
    out = np.zeros((SPC, 128, NPAD), np.float32)
    for s in range(SPC):
        img = np.zeros((64, H + 2, RW), np.float32)
        img[:, 1 : H + 1, 1 : W_DIM + 1] = x_pair[s]
        flat = img.reshape(64, -1)  # [64, 16900]
        xpad = np.zeros((64, NPAD), np.float32)
        xpad[:, IB:NPAD] = flat[:, : NPAD - IB]
        out[s, 0:64] = xpad
        out[s, 64:128, 0 : NPAD - RW] = xpad[:, RW:NPAD]
    return out.astype(ml_dtypes.bfloat16)


def _build_nc(split=True):
    import concourse.bass as bass
    import concourse.mybir as mybir
    import concourse.tile as tile

    f32 = mybir.dt.float32
    bf16 = mybir.dt.bfloat16
    RELU = mybir.ActivationFunctionType.Relu
    COPYF = mybir.ActivationFunctionType.Copy
    ADD = mybir.AluOpType.add
    MAX = mybir.AluOpType.max

    nc = bass.Bass(target_bir_lowering=False)

    xp = nc.declare_dram_parameter("xp", [SPC, 128, NPAD], bf16, isOutput=False)
    w2p = nc.declare_dram_parameter("w2p", [KDIM, NW2], bf16, isOutput=False)
    zt = nc.declare_dram_parameter("zt", [F_IN, SPC], bf16, isOutput=False)
    w0t = nc.declare_dram_parameter("w0t", [F_IN, 20], bf16, isOutput=False)
    b0 = nc.declare_dram_parameter("b0", [20, 1], f32, isOutput=False)
    w1t = nc.declare_dram_parameter("w1t", [20, 30], bf16, isOutput=False)
    b1 = nc.declare_dram_parameter("b1", [30, 1], f32, isOutput=False)
    hones = nc.declare_dram_parameter("hones", [4, 2 * SPC * 2], bf16, isOutput=False)
    y = nc.declare_dram_parameter("y", [SPC, C_OUT, H, W_DIM], bf16, isOutput=True)

    with tile.TileContext(nc) as tc:
        with (
            tc.tile_pool(name="persist", bufs=1) as pp,
            tc.tile_pool(name="mlp_ps", bufs=2, space="PSUM") as mlp_ps,
            tc.tile_pool(name="big_ps", bufs=6, space="PSUM") as big_ps,
            tc.tile_pool(name="tmp", bufs=4) as tmp_pool,
        ):
            # ---- front-loaded DMAs: small weights, w2p, then x (order
            # matters: the DMA engines are a serial resource at this size) --
            zt_sb = pp.tile([F_IN, SPC], bf16, tag="zt")
            w0t_sb = pp.tile([F_IN, 20], bf16, tag="w0t")
            b0_sb = pp.tile([20, 1], f32, tag="b0")
            w1t_sb = pp.tile([20, 30], bf16, tag="w1t")
            b1_sb = pp.tile([30, 1], f32, tag="b1")
            hones_sb = None
            for sb, dr in ((zt_sb, zt), (w0t_sb, w0t), (b0_sb, b0), (w1t_sb, w1t), (b1_sb, b1)):
                nc.sync.dma_start(out=sb[:, :], in_=dr[:, :])

            w2p_sb = pp.tile([KDIM, NW2], bf16, tag="w2p")
            w2p_inst = None
            for ck in range(4):
                lo, hi = ck * 3088, min(NW2, (ck + 1) * 3088)
                w2p_inst = nc.sync.dma_start(
                    out=w2p_sb[:, lo:hi], in_=w2p[:, lo:hi]
                )

            from concourse.tile_rust import add_dep_helper
            x2 = []
            x2_insts = []
            for s in range(SPC):
                t = pp.tile([128, NPAD], bf16, tag=f"x2_{s}")
                x2.append(t)
                xi = nc.sync.dma_start(out=t[:, :], in_=xp[s, :, :])
                x2_insts.append(xi)
                # keep the (startup-critical) w2p load ahead of the bulk x
                # loads on the shared DMA engines
                add_dep_helper(xi.ins, w2p_inst.ins, reason="x after w2p")

            # ---------- small MLP ----------
            ps0 = mlp_ps.tile([20, SPC], f32, tag="h")
            nc.tensor.matmul(ps0[:, :], w0t_sb[:, :], zt_sb[:, :], start=True, stop=True)
            h0_sb = pp.tile([20, SPC], bf16, tag="h0")
            nc.scalar.activation(out=h0_sb[:, :], in_=ps0[:, :], func=RELU, bias=b0_sb[:, :], scale=1.0)

            ps1 = mlp_ps.tile([30, SPC], f32, tag="h")
            nc.tensor.matmul(ps1[:, :], w1t_sb[:, :], h0_sb[:, :], start=True, stop=True)
            h1_sb = pp.tile([30, SPC], bf16, tag="h1")
            nc.scalar.activation(out=h1_sb[:, :], in_=ps1[:, :], func=RELU, bias=b1_sb[:, :], scale=1.0)

            # ---------- h4 block-diagonal [124, 8] (gpsimd: FIFO-ordered) ---
            h4 = pp.tile([KDIM, 2 * SPC * 2], bf16, tag="h4")
            nc.gpsimd.memset(h4[:, :], 0.0)
            for j in range(4):
                for s in range(SPC):
                    c = 4 * s + j
                    nc.gpsimd.dma_start(
                        out=h4[30 * j : 30 * j + 30, c : c + 1],
                        in_=h1_sb[0:30, s : s + 1],
                    )
            nc.gpsimd.dma_start(out=h4[120:124, :], in_=hones[:, :])

            # ---------- big GEMM; evacuate per dj-region on ONE engine so
            # each L DMA carries a single wait ----------
            aw_sb = pp.tile([8, NW2], bf16, tag="aw")
            evac_engine = {0: nc.scalar, 1: nc.vector}
            off = 0
            ti = 0
            while off < NW2:
                w = min(NTILE, NW2 - off)
                gp = big_ps.tile([128, NTILE], f32, tag="c")
                nc.tensor.matmul(
                    gp[0:8, 0:w], h4[:, :], w2p_sb[:, off : off + w],
                    start=True, stop=True,
                )
                eng = evac_engine[ti % 2]
                if eng is nc.scalar:
                    nc.scalar.activation(out=aw_sb[:, off : off + w], in_=gp[0:8, 0:w], func=RELU)
                else:
                    nc.vector.tensor_scalar(
                        out=aw_sb[:, off : off + w], in0=gp[0:8, 0:w],
                        scalar1=0.0, scalar2=None, op0=MAX,
                    )
                off += w
                ti += 1

            # ---------- L tiles: 24 contiguous SBUF->SBUF DMAs ----------
            ltiles = {}
            for s in range(SPC):
                for dj in range(3):
                    lt = pp.tile([128, 128], bf16, tag=f"L_{s}_{dj}")
                    ltiles[(s, dj)] = lt
                    for j in range(4):
                        last_l_dma = nc.sync.dma_start(
                            out=lt[32 * j : 32 * j + 32, :],
                            in_=aw_sb[4 * s + j : 4 * s + j + 1,
                                      dj * 4096 : (dj + 1) * 4096],
                        )
            bias_sb = []
            for s in range(SPC):
                bt = pp.tile([64, 1], f32, tag=f"bias_{s}")
                bias_sb.append(bt)
                nc.gpsimd.dma_start(
                    out=bt[:, :], in_=aw_sb[4 * s : 4 * s + 1, 12288:12352]
                )
            # sample-1 x load yields the DMA engines to the L-tile DMAs
            # (conv s0 start is gated on them; x_s1 isn't needed until later)
            add_dep_helper(x2_insts[1].ins, last_l_dma.ins, reason="x1 after L")

            # ---------- conv ----------
            with tc.tile_pool(name="out", bufs=1) as op, \
                 tc.tile_pool(name="tb", bufs=3) as tbp:
                for s in range(SPC):
                    out_sb = op.tile([64, NOUT + 4], bf16, tag=f"out_{s}")
                    psums = [None] * NG
                    tbs = [None] * NG

                    def merge(g, psums=psums, tbs=tbs, out_sb=out_sb, s=s):
                        base = NTILE * g
                        w0_ = min(NTILE, NOUT - base)
                        wa = min(w0_, SPLIT)
                        pg = psums[g]
                        pre = tmp_pool.tile([64, NTILE], f32, tag="pre")
                        nc.vector.tensor_add(
                            out=pre[:, 0:wa],
                            in0=pg[0:64, 0:wa],
                            in1=tbs[g][:, RW : RW + wa],
                        )
                        if w0_ > SPLIT:
                            nc.vector.tensor_add(
                                out=pre[:, SPLIT:w0_],
                                in0=pg[0:64, SPLIT:w0_],
                                in1=tbs[g + 1][:, 0 : w0_ - SPLIT],
                            )
                        if g % 4 == 3:
                            nc.scalar.activation(
                                out=out_sb[:, base : base + w0_],
                                in_=pre[:, 0:w0_], func=RELU,
                                bias=bias_sb[s][:, :], scale=1.0,
                            )
                        else:
                            nc.gpsimd.tensor_scalar(
                                out=out_sb[:, base : base + w0_],
                                in0=pre[:, 0:w0_],
                                scalar1=bias_sb[s][:, :], scalar2=0.0,
                                op0=ADD, op1=MAX,
                            )

                    done_rows = 0
                    for g in range(NG):
                        cp = big_ps.tile([128, NTILE], f32, tag="c")
                        psums[g] = cp
                        for dj in range(3):
                            q = NTILE * g + dj
                            nc.tensor.matmul(
                                cp[:, :], ltiles[(s, dj)][:, :],
                                x2[s][:, q : q + NTILE],
                                start=(dj == 0), stop=(dj == 2),
                            )
                        tb = tbp.tile([64, NTILE], f32, tag="tb")
                        tbs[g] = tb
                        nc.scalar.activation(out=tb[:, :], in_=cp[64:128, :], func=COPYF)
                        if g >= 1:
                            merge(g - 1)
                    merge(NG - 1)

                    for r0, r1 in Y_CHUNKS:
                        seg = out_sb[:, 1 + RW * r0 : 1 + RW * r1].rearrange(
                            "p (r c) -> p r c", c=RW
                        )[:, :, 0:W_DIM]
                        nc.sync.dma_start(out=y[s, :, r0:r1, :], in_=seg)

    if split:
        _split_multiwaits(nc)
    return nc


def _split_multiwaits(nc):
    """This container's walrus rejects instructions carrying more than one
    sync wait ("Too many sync wait commands").  Tile embeds the full wait set
    on each instruction, so split all-but-one wait out into standalone
    single-wait EventSemaphore instructions on the same engine, placed
    immediately before (engine streams are program-ordered within a block)."""
    import bass_rust

    n = 0
    for fn in nc.m.functions:
        for blk in fn.blocks:
            out = []
            for inst in blk.instructions:
                si = inst.sync_info
                waits = list(si.on_wait) if si is not None and si.on_wait else []
                if len(waits) > 1:
                    for w in waits[:-1]:
                        n += 1
                        ev = bass_rust.InstEventSemaphore(name=f"Wsplit-{n}")
                        ev.engine = inst.engine
                        ev.sync_info = bass_rust.SyncInfo(on_wait=[w], on_update=[])
                        out.append(ev)
                    inst.sync_info = bass_rust.SyncInfo(
                        on_wait=[waits[-1]], on_update=list(si.on_update or [])
                    )
                out.append(inst)
            blk.instructions = out


def _get_nc():
    global _NC_CACHE
    if _NC_CACHE is None:
        _NC_CACHE = _build_nc()
    return _NC_CACHE


def kernel(x, z, W0, b0, W1, b1, W2, b2):
    global LAST_EXEC_NS
    x = np.asarray(x, np.float32)
    z = np.asarray(z, np.float32)
    W0 = np.asarray(W0, np.float32)
    b0 = np.asarray(b0, np.float32)
    W1 = np.asarray(W1, np.float32)
    b1 = np.asarray(b1, np.float32)
    W2 = np.asarray(W2, np.float32)
    b2 = np.asarray(b2, np.float32)

    w2p_np = np.ascontiguousarray(_build_w2p(W2, b2).astype(ml_dtypes.bfloat16))
    w0t_np = np.ascontiguousarray(W0.T.astype(ml_dtypes.bfloat16))
    w1t_np = np.ascontiguousarray(W1.T.astype(ml_dtypes.bfloat16))
    b0_np = np.ascontiguousarray(b0.reshape(20, 1))
    b1_np = np.ascontiguousarray(b1.reshape(30, 1))
    hones_np = np.zeros((4, 8), np.float32)
    for j in range(4):
        hones_np[j, [j, 4 + j]] = 1.0
    hones_np = np.ascontiguousarray(hones_np.astype(ml_dtypes.bfloat16))

    in_maps = []
    for c in range(N_CORES):
        s0 = SPC * c
        in_maps.append({
            "xp": _build_x2_host(x[s0 : s0 + SPC]),
            "w2p": w2p_np,
            "zt": np.ascontiguousarray(z[s0 : s0 + SPC].T.astype(ml_dtypes.bfloat16)),
            "w0t": w0t_np,
            "b0": b0_np,
            "w1t": w1t_np,
            "b1": b1_np,
            "hones": hones_np,
        })

    from concourse.bass_utils import run_bass_kernel_spmd

    nc = _get_nc()
    res = run_bass_kernel_spmd(nc, in_maps, core_ids=list(range(N_CORES)), trace=TRACE)
    LAST_EXEC_NS = res.exec_time_ns
    out = np.concatenate([r["y"] for r in res.results], axis=0)
    return out.astype(np.float32)


# revision 29
# speedup vs baseline: 1.0023x; 1.0023x over previous
"""AdaptiveConv2d Trainium2 kernel (8 NeuronCores, data-parallel over batch).

Per core: 2 samples. On-device per sample:
  MLP z->20->30->36928 generates per-sample 3x3x64x64 conv kernels + biases,
  then 3x3 SAME conv (64->64ch, 128x128) + bias + relu.

Conv scheme ("M=128 row-pair packing", 75% PE util):
  x is staged host-side into a zero-padded (130-wide rows) stacked bf16
  layout X2 = [xpad ; xpad shifted +130] on 128 partitions (one contiguous
  full-width DMA per sample).  For each 512-pixel output tile g and each
  dj in {0,1,2} one bf16 matmul with lhsT L_dj [128,128] covers taps
  (di=-1,dj),(di=0,dj) for tile g (PSUM rows 0:64) and tap (di=+1,dj) for
  tile positions g*512-130 (PSUM rows 64:128).
  Merge pipeline (one 512-wide pass per engine per group):
    ACT:    fold-copy PSUM[64:128] -> SBUF tb (cross-partition copy)
    DVE:    out_pre = PSUM[0:64] + tb window(s)
    GPSIMD: out_sb = max(out_pre + bias, 0) (tensor_scalar, bf16 out)

MLP big layer runs as one bf16 GEMM with host-prepermuted W2 so that the
output lands in SBUF in a layout from which the six lhsT tiles (2 samples x
3 dj) are built with 24 contiguous SBUF->SBUF DMAs.

NOTE: this container's walrus accepts at most ONE sync wait per
instruction; _split_multiwaits() legalizes the Tile-scheduled module.
"""

import os
import numpy as np
import ml_dtypes

B, C_IN, C_OUT, KS, H, W_DIM, F_IN = 16, 64, 64, 3, 128, 128, 16
NKW = C_OUT * C_IN * KS * KS  # 36864
N_CORES = 8
SPC = B // N_CORES  # samples per core = 2

RW = W_DIM + 2          # padded row width = 130
NPAD = RW * (H + 2)     # 16900 padded positions (image base IB=1)
IB = 1
OB = IB + RW            # first output position = 131
NOUT = 16640            # out_sb width (indices 0..16638 used)
NTILE = 512
NG = 33
SPLIT = NTILE - RW      # 382
NW2 = 12352             # GEMM width: 3*32*128 kernels + 64 biases
KDIM = 124              # 4 chunks x 30 + 4 bias rows
Y_CHUNKS = [(0, 31), (31, 63), (63, 94), (94, 128)]  # rows per output DMA

LAST_EXEC_NS = None
TRACE = os.environ.get("BASS_KERNEL_TRACE", "0") == "1"

_NC_CACHE = None


def _build_w2p(W2, b2):
    """[124, 12352] f32: pre-permuted W2.T so GEMM output = L-tile layout."""
    W2P = np.zeros((KDIM, NW2), np.float32)
    dj, pi, mb, co = np.meshgrid(
        np.arange(3), np.arange(32), np.arange(2), np.arange(64), indexing="ij"
    )
    n = (dj * 32 + pi) * 128 + mb * 64 + co
    for j in range(4):
        p = 32 * j + pi
        kb = p // 64
        ci = p % 64
        di = np.where(mb == 0, kb, 2)
        valid = ~((mb == 1) & (kb == 0))
        m = co * 576 + ci * 9 + di * 3 + dj
        nv = n[valid]
        mv = m[valid]
        W2P[30 * j : 30 * j + 30, nv] = W2[mv, :].T
        W2P[120 + j, nv] = b2[mv]
    cot = np.arange(64)
    W2P[0:30, 12288 + cot] = W2[36864 + cot, :].T
    W2P[120, 12288 + cot] = b2[36864 + cot]
    return W2P


def _build_x2_host(x_pair):
    """[SPC, 128, NPAD] bf16: padded image + its (+130)-shifted copy stacked
    on the partition axis, zeros baked in."""
    out = np.zeros((SPC, 128, NPAD), np.float32)
    for s in range(SPC):
        img = np.zeros((64, H + 2, RW), np.float32)
        img[:, 1 : H + 1, 1 : W_DIM + 1] = x_pair[s]
        flat = img.reshape(64, -1)  # [64, 16900]
        xpad = np.zeros((64, NPAD), np.float32)
        xpad[:, IB:NPAD] = flat[:, : NPAD - IB]
        out[s, 0:64] = xpad
        out[s, 64:128, 0 : NPAD - RW] = xpad[:, RW:NPAD]
    return out.astype(ml_dtypes.bfloat16)


def _build_nc(split=True):
    import concourse.bass as bass
    import concourse.mybir as mybir
    import concourse.tile as tile

    f32 = mybir.dt.float32
    bf16 = mybir.dt.bfloat16
    RELU = mybir.ActivationFunctionType.Relu
    COPYF = mybir.ActivationFunctionType.Copy
    ADD = mybir.AluOpType.add
    MAX = mybir.AluOpType.max

    nc = bass.Bass(target_bir_lowering=False)

    xp = nc.declare_dram_parameter("xp", [SPC, 128, NPAD], bf16, isOutput=False)
    w2p = nc.declare_dram_parameter("w2p", [KDIM, NW2], bf16, isOutput=False)
    zt = nc.declare_dram_parameter("zt", [F_IN, SPC], bf16, isOutput=False)
    w0t = nc.declare_dram_parameter("w0t", [F_IN, 20], bf16, isOutput=False)
    b0 = nc.declare_dram_parameter("b0", [20, 1], f32, isOutput=False)
    w1t = nc.declare_dram_parameter("w1t", [20, 30], bf16, isOutput=False)
    b1 = nc.declare_dram_parameter("b1", [30, 1], f32, isOutput=False)
    hones = nc.declare_dram_parameter("hones", [4, 2 * SPC * 2], bf16, isOutput=False)
    y = nc.declare_dram_parameter("y", [SPC, C_OUT, H, W_DIM], bf16, isOutput=True)

    with tile.TileContext(nc) as tc:
        with (
            tc.tile_pool(name="persist", bufs=1) as pp,
            tc.tile_pool(name="mlp_ps", bufs=1, space="PSUM") as mlp_ps,
            tc.tile_pool(name="big_ps", bufs=7, space="PSUM") as big_ps,
            tc.tile_pool(name="tmp", bufs=4) as tmp_pool,
        ):
            # ---- front-loaded DMAs: small weights, w2p, then x (order
            # matters: the DMA engines are a serial resource at this size) --
            zt_sb = pp.tile([F_IN, SPC], bf16, tag="zt")
            w0t_sb = pp.tile([F_IN, 20], bf16, tag="w0t")
            b0_sb = pp.tile([20, 1], f32, tag="b0")
            w1t_sb = pp.tile([20, 30], bf16, tag="w1t")
            b1_sb = pp.tile([30, 1], f32, tag="b1")
            hones_sb = None
            for sb, dr in ((zt_sb, zt), (w0t_sb, w0t), (b0_sb, b0), (w1t_sb, w1t), (b1_sb, b1)):
                nc.sync.dma_start(out=sb[:, :], in_=dr[:, :])

            w2p_sb = pp.tile([KDIM, NW2], bf16, tag="w2p")
            w2p_inst = None
            for ck in range(4):
                lo, hi = ck * 3088, min(NW2, (ck + 1) * 3088)
                w2p_inst = nc.sync.dma_start(
                    out=w2p_sb[:, lo:hi], in_=w2p[:, lo:hi]
                )

            from concourse.tile_rust import add_dep_helper
            x2 = []
            x2_insts = []
            for s in range(SPC):
                t = pp.tile([128, NPAD], bf16, tag=f"x2_{s}")
                x2.append(t)
                xi = nc.sync.dma_start(out=t[:, :], in_=xp[s, :, :])
                x2_insts.append(xi)
                # keep the (startup-critical) w2p load ahead of the bulk x
                # loads on the shared DMA engines
                add_dep_helper(xi.ins, w2p_inst.ins, reason="x after w2p")

            # ---------- small MLP ----------
            ps0 = mlp_ps.tile([20, SPC], f32, tag="h")
            nc.tensor.matmul(ps0[:, :], w0t_sb[:, :], zt_sb[:, :], start=True, stop=True)
            h0_sb = pp.tile([20, SPC], bf16, tag="h0")
            nc.scalar.activation(out=h0_sb[:, :], in_=ps0[:, :], func=RELU, bias=b0_sb[:, :], scale=1.0)

            ps1 = mlp_ps.tile([30, SPC], f32, tag="h")
            nc.tensor.matmul(ps1[:, :], w1t_sb[:, :], h0_sb[:, :], start=True, stop=True)
            h1_sb = pp.tile([30, SPC], bf16, tag="h1")
            nc.scalar.activation(out=h1_sb[:, :], in_=ps1[:, :], func=RELU, bias=b1_sb[:, :], scale=1.0)

            # ---------- h4 block-diagonal [124, 8] (gpsimd: FIFO-ordered) ---
            h4 = pp.tile([KDIM, 2 * SPC * 2], bf16, tag="h4")
            nc.gpsimd.memset(h4[:, :], 0.0)
            for j in range(4):
                for s in range(SPC):
                    c = 4 * s + j
                    nc.gpsimd.dma_start(
                        out=h4[30 * j : 30 * j + 30, c : c + 1],
                        in_=h1_sb[0:30, s : s + 1],
                    )
            nc.gpsimd.dma_start(out=h4[120:124, :], in_=hones[:, :])

            # ---------- big GEMM; evacuate per dj-region on ONE engine so
            # each L DMA carries a single wait ----------
            aw_sb = pp.tile([8, NW2], bf16, tag="aw")
            evac_engine = {0: nc.scalar, 1: nc.vector}
            off = 0
            ti = 0
            while off < NW2:
                w = min(NTILE, NW2 - off)
                gp = big_ps.tile([128, NTILE], f32, tag="c")
                nc.tensor.matmul(
                    gp[0:8, 0:w], h4[:, :], w2p_sb[:, off : off + w],
                    start=True, stop=True,
                )
                eng = evac_engine[ti % 2]
                if eng is nc.scalar:
                    nc.scalar.activation(out=aw_sb[:, off : off + w], in_=gp[0:8, 0:w], func=RELU)
                else:
                    nc.vector.tensor_scalar(
                        out=aw_sb[:, off : off + w], in0=gp[0:8, 0:w],
                        scalar1=0.0, scalar2=None, op0=MAX,
                    )
                off += w
                ti += 1

            # ---------- L tiles: 24 contiguous SBUF->SBUF DMAs ----------
            ltiles = {}
            for s in range(SPC):
                for dj in range(3):
                    lt = pp.tile([128, 128], bf16, tag=f"L_{s}_{dj}")
                    ltiles[(s, dj)] = lt
                    for j in range(4):
                        last_l_dma = nc.sync.dma_start(
                            out=lt[32 * j : 32 * j + 32, :],
                            in_=aw_sb[4 * s + j : 4 * s + j + 1,
                                      dj * 4096 : (dj + 1) * 4096],
                        )
            bias_sb = []
            for s in range(SPC):
                bt = pp.tile([64, 1], f32, tag=f"bias_{s}")
                bias_sb.append(bt)
                nc.gpsimd.dma_start(
                    out=bt[:, :], in_=aw_sb[4 * s : 4 * s + 1, 12288:12352]
                )
            # sample-1 x load yields the DMA engines to the L-tile DMAs
            # (conv s0 start is gated on them; x_s1 isn't needed until later)
            add_dep_helper(x2_insts[1].ins, last_l_dma.ins, reason="x1 after L")

            # ---------- conv ----------
            with tc.tile_pool(name="out", bufs=1) as op, \
                 tc.tile_pool(name="tb", bufs=4) as tbp:
                for s in range(SPC):
                    out_sb = op.tile([64, NOUT + 4], bf16, tag=f"out_{s}")
                    psums = [None] * NG
                    tbs = [None] * NG

                    def merge(g, psums=psums, tbs=tbs, out_sb=out_sb, s=s):
                        base = NTILE * g
                        w0_ = min(NTILE, NOUT - base)
                        wa = min(w0_, SPLIT)
                        pg = psums[g]
                        pre = tmp_pool.tile([64, NTILE], f32, tag="pre")
                        nc.vector.tensor_add(
                            out=pre[:, 0:wa],
                            in0=pg[0:64, 0:wa],
                            in1=tbs[g][:, RW : RW + wa],
                        )
                        if w0_ > SPLIT:
                            nc.vector.tensor_add(
                                out=pre[:, SPLIT:w0_],
                                in0=pg[0:64, SPLIT:w0_],
                                in1=tbs[g + 1][:, 0 : w0_ - SPLIT],
                            )
                        if g % 4 == 3:
                            nc.scalar.activation(
                                out=out_sb[:, base : base + w0_],
                                in_=pre[:, 0:w0_], func=RELU,
                                bias=bias_sb[s][:, :], scale=1.0,
                            )
                        else:
                            nc.gpsimd.tensor_scalar(
                                out=out_sb[:, base : base + w0_],
                                in0=pre[:, 0:w0_],
                                scalar1=bias_sb[s][:, :], scalar2=0.0,
                                op0=ADD, op1=MAX,
                            )

                    done_rows = 0
                    for g in range(NG):
                        cp = big_ps.tile([128, NTILE], f32, tag="c")
                        psums[g] = cp
                        for dj in range(3):
                            q = NTILE * g + dj
                            nc.tensor.matmul(
                                cp[:, :], ltiles[(s, dj)][:, :],
                                x2[s][:, q : q + NTILE],
                                start=(dj == 0), stop=(dj == 2),
                            )
                        tb = tbp.tile([64, NTILE], f32, tag="tb")
                        tbs[g] = tb
                        nc.scalar.activation(out=tb[:, :], in_=cp[64:128, :], func=COPYF)
                        if g >= 1:
                            merge(g - 1)
                    merge(NG - 1)

                    for r0, r1 in Y_CHUNKS:
                        seg = out_sb[:, 1 + RW * r0 : 1 + RW * r1].rearrange(
                            "p (r c) -> p r c", c=RW
                        )[:, :, 0:W_DIM]
                        nc.sync.dma_start(out=y[s, :, r0:r1, :], in_=seg)

    if split:
        _split_multiwaits(nc)
    return nc


def _split_multiwaits(nc):
    """This container's walrus rejects instructions carrying more than one
    sync wait ("Too many sync wait commands").  Tile embeds the full wait set
    on each instruction, so split all-but-one wait out into standalone
    single-wait EventSemaphore instructions on the same engine, placed
    immediately before (engine streams are program-ordered within a block)."""
    import bass_rust

    n = 0
    for fn in nc.m.functions:
        for blk in fn.blocks:
            out = []
            for inst in blk.instructions:
                si = inst.sync_info
                waits = list(si.on_wait) if si is not None and si.on_wait else []
                if len(waits) > 1:
                    for w in waits[:-1]:
                        n += 1
                        ev = bass_rust.InstEventSemaphore(name=f"Wsplit-{n}")
                        ev.engine = inst.engine
                        ev.sync_info = bass_rust.SyncInfo(on_wait=[w], on_update=[])
                        out.append(ev)
                    inst.sync_info = bass_rust.SyncInfo(
                        on_wait=[waits[-1]], on_update=list(si.on_update or [])
                    )
                out.append(inst)
            blk.instructions = out


def _get_nc():
    global _NC_CACHE
    if _NC_CACHE is None:
        _NC_CACHE = _build_nc()
    return _NC_CACHE


def kernel(x, z, W0, b0, W1, b1, W2, b2):
    global LAST_EXEC_NS
    x = np.asarray(x, np.float32)
    z = np.asarray(z, np.float32)
    W0 = np.asarray(W0, np.float32)
    b0 = np.asarray(b0, np.float32)
    W1 = np.asarray(W1, np.float32)
    b1 = np.asarray(b1, np.float32)
    W2 = np.asarray(W2, np.float32)
    b2 = np.asarray(b2, np.float32)

    w2p_np = np.ascontiguousarray(_build_w2p(W2, b2).astype(ml_dtypes.bfloat16))
    w0t_np = np.ascontiguousarray(W0.T.astype(ml_dtypes.bfloat16))
    w1t_np = np.ascontiguousarray(W1.T.astype(ml_dtypes.bfloat16))
    b0_np = np.ascontiguousarray(b0.reshape(20, 1))
    b1_np = np.ascontiguousarray(b1.reshape(30, 1))
    hones_np = np.zeros((4, 8), np.float32)
    for j in range(4):
        hones_np[j, [j, 4 + j]] = 1.0
    hones_np = np.ascontiguousarray(hones_np.astype(ml_dtypes.bfloat16))

    in_maps = []
    for c in range(N_CORES):
        s0 = SPC * c
        in_maps.append({
            "xp": _build_x2_host(x[s0 : s0 + SPC]),
            "w2p": w2p_np,
            "zt": np.ascontiguousarray(z[s0 : s0 + SPC].T.astype(ml_dtypes.bfloat16)),
            "w0t": w0t_np,
            "b0": b0_np,
            "w1t": w1t_np,
            "b1": b1_np,
            "hones": hones_np,
        })

    from concourse.bass_utils import run_bass_kernel_spmd

    nc = _get_nc()
    res = run_bass_kernel_spmd(nc, in_maps, core_ids=list(range(N_CORES)), trace=TRACE)
    LAST_EXEC_NS = res.exec_time_ns
    out = np.concatenate([r["y"] for r in res.results], axis=0)
    return out.astype(np.float32)
